# revision 45
# baseline (speedup 1.0000x reference)
"""Trainium2 Bass kernel for nn_MicroCoupledSuperNet (GNN message passing supernet).

Strategy (8-core SPMD, dst-node sharding), v2:
  - Layer 1 aggregation inputs are HOST-GATHERED: the per-edge source rows of x
    are pre-permuted on the host into the exact [128-edge-slot, tile] stream the
    aggregation matmuls consume, so layer 1 runs on fast contiguous DMA with no
    runtime gathers at all.
  - Layer 2 gathers h1 rows at runtime via SWDGE dma_gather, with buckets merged
    to near the 1024-descriptor ring limit (pair-of-blocks x table-half) to
    amortize the ~0.9us per-instruction cost, and descriptor loads balanced
    across the 4 SWDGE queues.
  - h1 is exchanged via a 3-chunk AllGather issued as superblocks complete, so
    most of the collective hides under layer-1 compute.  h1f uses a chunk-major
    row layout; gather indices are remapped accordingly on the host.
  - Aggregation accumulates GCN|SAGE columns of a block PAIR in one [128,256]
    PSUM tile; the Scalar engine (not Vector) drains PSUM into bf16 operands
    reordered as [g0 g1 | s0 s1] for the dense stage.
  - Dense conv-mix is 3 matmuls per 128-node pair + fused LayerNorm-mix and
    activation-mix chain; sum-pool readout is a 0/1 matmul into per-core graph
    slots; host merges windows and adds post_b.
"""

import sys
import math
import dataclasses

import numpy as np

for _p in ("/opt/trn_rl_repo",):
    if _p not in sys.path:
        sys.path.insert(0, _p)

import ml_dtypes  # noqa: E402

BF16 = ml_dtypes.bfloat16

from concourse import bass, bacc, mybir, tile  # noqa: E402
from concourse.bass_utils import run_bass_kernel_spmd  # noqa: E402

P = 128          # SBUF partitions / edge-tile rows
BLK = 64         # destination nodes per aggregation block
H = 128          # hidden dim (== D_IN)
DOUT = 64
SBLK = 8         # aggregation blocks per superblock (scheduling unit)
GSLOTS = 128     # per-core graph slots for pooling
EPS = 1e-5
HALF = 32768     # int16 gather-table split
MAXG = 1024      # SWDGE ring limit (descs per gather)
F32 = mybir.dt.float32
BF = mybir.dt.bfloat16
I16 = mybir.dt.int16

# AllGather chunk boundaries, in superblock units (inclusive end sb index).
# Chunk 0 ends at row 4096 so its h1f positions are exactly [0, 32768) == the
# int16 gather table's low half: half-0 gathers depend only on chunk 0.
AG_SB_END = [7, 12]


@dataclasses.dataclass
class Cfg:
    N: int
    E: int
    G: int
    cores: int
    nshard: int = 0
    nblk: int = 0
    npair: int = 0
    npad: int = 0
    nsb: int = 0

    def __post_init__(self):
        assert self.N % self.cores == 0
        self.nshard = self.N // self.cores
        self.nblk = math.ceil(self.nshard / BLK)
        if self.nblk % 2:
            self.nblk += 1  # keep whole pairs
        self.npair = self.nblk // 2
        self.npad = self.nblk * BLK
        self.nsb = math.ceil(self.nblk / SBLK)


def _softmax(v):
    v = np.asarray(v, np.float64)
    e = np.exp(v - v.max())
    return e / e.sum()


def _ceil16(x):
    return (int(x) + 15) // 16 * 16


@dataclasses.dataclass
class Sched:
    # layer 1 (host-gathered stream, per-block tiles, pair-major)
    T1: np.ndarray           # [nblk] tiles per block
    ecol1: list              # per block: stream col offset
    ecols1: int
    tpb1: np.ndarray         # [npair] tiles per pair
    # layer 2 (runtime gather)
    buckets: list            # per pair: list of gather segs (dicts)
    mmt: list                # per pair: list of (side, kk, r0, r1) mm chunks
    ecol2: list              # per pair: est2 col offset
    ecols2: int
    idx_cols: int
    tpb2: np.ndarray         # [npair] tiles per pair
    # scalar constants per layer
    wc: np.ndarray
    wn: np.ndarray
    wa: np.ndarray
    have_bias1: bool = True
    have_bias2: bool = True
    have_lnb: list = None
    shard_rows: int = 0
    ag_rows: list = None     # [(row_start, row_end, pos_base)] per AG chunk


def host_prep(inputs: dict, cfg: Cfg):
    x = np.asarray(inputs["x"], np.float32)
    ei = np.asarray(inputs["edge_index"])
    batch = np.asarray(inputs["batch"]).astype(np.int64)
    src = ei[0].astype(np.int64)
    dst = ei[1].astype(np.int64)
    N, E, G_N, C = cfg.N, cfg.E, cfg.G, cfg.cores
    ns = cfg.nshard

    deg_sl = np.bincount(dst, minlength=N).astype(np.float64) + 1.0
    dinv = 1.0 / np.sqrt(deg_sl)
    degn = np.maximum(np.bincount(dst, minlength=N), 1).astype(np.float64)

    # ---- AllGather chunk layout + h1f position remap (chunk-major) ----
    sb_rows = [min((e + 1) * SBLK * BLK, ns) for e in AG_SB_END]
    chunk_rs = [0] + sb_rows[:-1]
    chunk_re = sb_rows
    chunk_len = [e - s for s, e in zip(chunk_rs, chunk_re)]
    pos_base = [0]
    for ln in chunk_len[:-1]:
        pos_base.append(pos_base[-1] + C * ln)
    assert pos_base[-1] + C * chunk_len[-1] == N
    posmap = np.empty(N, np.int64)
    for o in range(C):
        for k in range(len(chunk_len)):
            rs, re, ln, pb = chunk_rs[k], chunk_re[k], chunk_len[k], pos_base[k]
            posmap[o * ns + rs:o * ns + re] = pb + o * ln + np.arange(ln)
    ag_rows = [(chunk_rs[k], chunk_re[k], pos_base[k])
               for k in range(len(chunk_len))]

    # ---- per-core edge lists (with self-loop pseudo-edges) ----
    per_core = []
    cnt1 = np.zeros((C, cfg.nblk), np.int64)
    cnt2 = np.zeros((C, cfg.nblk, 2), np.int64)
    for c in range(C):
        lo, hi = c * ns, (c + 1) * ns
        m = (dst >= lo) & (dst < hi)
        es, ed = src[m], dst[m]
        dd = np.arange(lo, hi, dtype=np.int64)
        asrc = np.concatenate([es, dd])
        adst = np.concatenate([ed, dd])
        wg = np.concatenate([dinv[es] * dinv[ed], dinv[dd] ** 2])
        ws = np.concatenate([1.0 / degn[ed], np.zeros(ns)])
        dloc = adst - lo
        blk = dloc // BLK
        din = dloc % BLK
        psrc = posmap[asrc]
        hf = (psrc >= HALF).astype(np.int64)
        np.add.at(cnt1[c], blk, 1)
        for b in range(cfg.nblk):
            mb = blk == b
            cnt2[c, b, 0] = int((mb & (hf == 0)).sum())
            cnt2[c, b, 1] = int((mb & (hf == 1)).sum())
        per_core.append((asrc, psrc, wg, ws, blk, din, hf))

    # ---- layer-1 schedule: per-block tiles, pair-major contiguous ----
    T1 = np.ceil(cnt1.max(axis=0) / P).astype(np.int64)
    ecol1 = []
    off = 0
    for b in range(cfg.nblk):
        ecol1.append(off)
        off += int(T1[b]) * P
    ecols1 = off
    tpb1 = np.array([int(T1[2 * p] + T1[2 * p + 1]) for p in range(cfg.npair)])

    # ---- layer-2 schedule: (pair, half) gather buckets, 16-granular ----
    # Within a bucket the two blocks' slot runs are packed back to back at
    # 16-slot granularity (uniform across cores via max counts); the tile that
    # straddles the block boundary is consumed by two partition-sliced matmuls.
    mx2 = cnt2.max(axis=0)                      # [nblk, 2]
    qload = [0, 0, 0, 0]
    buckets = []         # per pair: list of gather segs
    mmt = []             # per pair: list of (side, kk, r0, r1) matmul chunks
    ecol2 = []
    off = 0              # stream cols
    tpb2 = np.zeros(cfg.npair, np.int64)
    idx_off = 0
    for p in range(cfg.npair):
        ecol2.append(off)
        segs = []
        runs = []
        toff = 0
        for h in (0, 1):
            b0, b1 = 2 * p, 2 * p + 1
            # first block's span rounds to a full tile: straddle-free matmuls
            M0 = (int(mx2[b0, h]) + P - 1) // P * P
            M1 = _ceil16(mx2[b1, h])
            tot = M0 + M1
            if tot == 0:
                continue
            if tot <= MAXG:
                ntl = (tot + P - 1) // P
                segs.append({"tiles": (toff, ntl), "n": tot, "h": h,
                             "blocks": [(b0, 0), (b1, M0)]})
                ts = M0 // P
                for k in range(ts):
                    runs.append((0, toff + k, 0, P))
                for k in range(ts, ntl):
                    runs.append((1, toff + k, 0, P))
                toff += ntl
            else:
                # fallback: two tile-aligned gathers
                for b, M in ((b0, M0), (b1, M1)):
                    if M == 0:
                        continue
                    assert M <= MAXG
                    ntl = (M + P - 1) // P
                    segs.append({"tiles": (toff, ntl), "n": M, "h": h,
                                 "blocks": [(b, 0)]})
                    for k in range(ntl):
                        runs.append((b - 2 * p, toff + k, 0, P))
                    toff += ntl
        for s in segs:
            q = min(range(4), key=lambda i: qload[i])
            qload[q] += s["n"]
            s["q"] = q
            s["io"] = idx_off
            idx_off += s["n"]
        buckets.append(segs)
        # side-major order: keep each side's PSUM accumulation group contiguous
        runs = ([r for r in runs if r[0] == 0] + [r for r in runs if r[0] == 1])
        mmt.append(runs)
        tpb2[p] = toff
        off += toff * P
    ecols2 = off
    assert idx_off % 16 == 0
    idx_cols = idx_off // 16
    idx_layout = list(zip(range(cfg.npair), buckets))

    # ---- pack per-core streams ----
    data = []
    for c in range(C):
        asrc, psrc, wg, ws, blk, din, hf = per_core[c]
        # order edges by (block, half) and get positions within each group
        key = blk * 2 + hf
        order = np.argsort(key, kind="stable")
        asrc, psrc, wg, ws, blk, din, hf = (
            a[order] for a in (asrc, psrc, wg, ws, blk, din, hf))
        pos2 = np.zeros(len(asrc), np.int64)
        start = 0
        for b in range(cfg.nblk):
            for h in (0, 1):
                nbh = int(cnt2[c, b, h])
                pos2[start:start + nbh] = np.arange(nbh)
                start += nbh
        # layer-1 positions: within block (ignore halves)
        order1 = np.argsort(blk, kind="stable")
        pos1 = np.zeros(len(asrc), np.int64)
        start = 0
        for b in range(cfg.nblk):
            nb_ = int(cnt1[c, b])
            pos1[start:start + nb_] = np.arange(nb_)
            start += nb_
        # scatter into layer-1 streams
        n_t1 = int(T1.sum())
        Xfull = np.zeros((n_t1, P, H), np.float32)
        E1full = np.zeros((n_t1, P, P), np.float32)
        b1 = blk[order1]
        d1 = din[order1]
        s1 = asrc[order1]
        wg1 = wg[order1]
        ws1 = ws[order1]
        tbase1 = np.array([ecol1[b] // P for b in range(cfg.nblk)])
        tno1 = tbase1[b1] + pos1 // P
        prow1 = pos1 % P
        Xfull[tno1, prow1, :] = x[s1]
        E1full[tno1, prow1, d1] = wg1
        E1full[tno1, prow1, BLK + d1] = ws1
        xg = np.ascontiguousarray(
            Xfull.transpose(1, 0, 2).reshape(P, n_t1 * H)).astype(BF16)
        est1 = np.ascontiguousarray(
            E1full.transpose(1, 0, 2).reshape(P, n_t1 * P)).astype(BF16)
        # layer-2 E stream + idx stream
        n_t2 = int(tpb2.sum())
        E2full = np.zeros((n_t2, P, P), np.float32)
        idxflat = np.zeros(idx_off, np.int64)
        sb2 = np.zeros((cfg.nblk, 2), np.int64)     # global slot base
        ib2 = np.zeros((cfg.nblk, 2), np.int64)     # idx position base
        for pp, segs in idx_layout:
            for s in segs:
                for (b, boff) in s["blocks"]:
                    sb2[b, s["h"]] = (ecol2[pp] // P + s["tiles"][0]) * P + boff
                    ib2[b, s["h"]] = s["io"] + boff
        gslot = sb2[blk, hf] + pos2
        tno2 = gslot // P
        prow2 = gslot % P
        E2full[tno2, prow2, din] = wg
        E2full[tno2, prow2, BLK + din] = ws
        est2 = np.ascontiguousarray(
            E2full.transpose(1, 0, 2).reshape(P, n_t2 * P)).astype(BF16)
        ipos = ib2[blk, hf] + pos2
        idxflat[ipos] = psrc - hf * HALF
        wrapped = idxflat.reshape(-1, 16).T
        idx16 = np.tile(wrapped, (8, 1)).astype(np.int16)
        assert idx16.shape[1] == idx_cols
        data.append({"xg": xg, "est1": est1, "est2": est2, "idx": idx16})

    # ---- pooling ----
    g_lo = []
    for c in range(C):
        lo = int(batch[c * ns])
        hi = int(batch[(c + 1) * ns - 1])
        span = hi - lo + 1
        assert span <= GSLOTS, f"graph span {span} exceeds {GSLOTS}"
        g_lo.append(lo)
        ep = np.zeros((cfg.npad, GSLOTS), np.float32)
        rows = np.arange(ns)
        ep[rows, batch[c * ns:(c + 1) * ns] - lo] = 1.0
        epm = np.ascontiguousarray(
            ep.reshape(cfg.npair, P, GSLOTS).transpose(1, 0, 2)
            .reshape(P, cfg.npair * GSLOTS)).astype(BF16)
        data[c]["epool"] = epm

    # ---- weights / constants (identical to v1) ----
    pre_w = np.asarray(inputs["pre_w"], np.float64)
    pre_b = np.asarray(inputs["pre_b"], np.float64)
    post_w = np.asarray(inputs["post_w"], np.float64)
    post_b = np.asarray(inputs["post_b"], np.float64)
    gcn_w = np.asarray(inputs["gcn_w"], np.float64)
    gcn_b = np.asarray(inputs["gcn_b"], np.float64)
    sage_ws = np.asarray(inputs["sage_ws"], np.float64)
    sage_wn = np.asarray(inputs["sage_wn"], np.float64)
    ln_g = np.asarray(inputs["ln_g"], np.float64)
    ln_b = np.asarray(inputs["ln_b"], np.float64)
    a_conv = np.asarray(inputs["a_conv"], np.float64)
    a_norm = np.asarray(inputs["a_norm"], np.float64)
    a_act = np.asarray(inputs["a_act"], np.float64)

    wc = np.stack([_softmax(a_conv[l]) for l in range(2)])
    wn = np.stack([_softmax(a_norm[l]) for l in range(2)])
    wa = np.stack([_softmax(a_act[l]) for l in range(2)])

    Vg1 = pre_w @ (wc[0, 0] * gcn_w[0])
    VI1 = pre_w @ (wc[0, 1] * sage_ws[0])
    Vs1 = pre_w @ (wc[0, 1] * sage_wn[0])
    Vg2 = wc[1, 0] * gcn_w[1]
    VI2 = wc[1, 1] * sage_ws[1]
    Vs2 = wc[1, 1] * sage_wn[1]
    vm = np.stack([Vg1, VI1, Vs1, Vg2, VI2, Vs2]).astype(BF16)

    qg = wc[0, 0] * (pre_b @ gcn_w[0])
    qs = wc[0, 1] * (pre_b @ sage_wn[0])
    qc = wc[0, 0] * gcn_b[0] + wc[0, 1] * (pre_b @ sage_ws[0])
    bc2 = wc[1, 0] * gcn_b[1]
    qv = np.stack([qg, qs, qc, bc2]).astype(BF16)
    have_bias1 = bool(np.abs(qv[:3]).max() > 0)
    have_bias2 = bool(np.abs(bc2).max() > 0)

    rs_gcn_full = np.zeros(N)
    np.add.at(rs_gcn_full, dst, dinv[src])
    rs_gcn_full = dinv * rs_gcn_full + dinv ** 2
    rs_sage_full = (np.bincount(dst, minlength=N) > 0).astype(np.float64)
    for c in range(C):
        r = np.zeros((3, cfg.npad), np.float32)
        r[0, :ns] = rs_gcn_full[c * ns:(c + 1) * ns]
        r[1, :ns] = rs_sage_full[c * ns:(c + 1) * ns]
        r[2, :] = 1.0
        data[c]["rsv"] = r.astype(BF16)

    G1 = wn[0, 0] * ln_g[0]
    B1 = wn[0, 0] * ln_b[0]
    G2 = wn[1, 0] * ln_g[1]
    B2 = wn[1, 0] * ln_b[1]
    lnm = np.stack([np.tile(G1, (P, 1)), np.tile(B1, (P, 1)),
                    np.tile(G2, (P, 1)), np.tile(B2, (P, 1))]).astype(np.float32)
    have_lnb = [bool(np.abs(B1).max() > 0), bool(np.abs(B2).max() > 0)]

    for c in range(C):
        xs = np.zeros((cfg.npad, H), np.float32)
        xs[:ns] = x[c * ns:(c + 1) * ns]
        data[c]["xst"] = np.ascontiguousarray(xs.T).astype(BF16)
        data[c]["vm"] = vm
        data[c]["qv"] = qv
        data[c]["lnm"] = lnm
        data[c]["pw"] = post_w.astype(BF16)
        data[c]["ident"] = np.eye(P, dtype=np.float32).astype(BF16)

    sched = Sched(T1=T1, ecol1=ecol1, ecols1=ecols1, tpb1=tpb1,
                  buckets=buckets, mmt=mmt, ecol2=ecol2,
                  ecols2=ecols2, idx_cols=idx_cols,
                  tpb2=tpb2,
                  wc=wc, wn=wn, wa=wa,
                  have_bias1=have_bias1, have_bias2=have_bias2,
                  have_lnb=have_lnb, shard_rows=ns, ag_rows=ag_rows)
    combine = {"g_lo": g_lo, "post_b": post_b}
    return sched, data, combine


def build_program(cfg: Cfg, sched: Sched):
    nc = bacc.Bacc("TRN2", target_bir_lowering=False, debug=False,
                   enable_asserts=False, num_devices=cfg.cores,
                   num_swdge_queues=4)

    xg_d = nc.dram_tensor("xg", [P, sched.ecols1], BF, kind="ExternalInput")
    est1_d = nc.dram_tensor("est1", [P, sched.ecols1], BF, kind="ExternalInput")
    est2_d = nc.dram_tensor("est2", [P, sched.ecols2], BF, kind="ExternalInput")
    idx_d = nc.dram_tensor("idx", [P, sched.idx_cols], I16, kind="ExternalInput")
    epool_d = nc.dram_tensor("epool", [P, cfg.npair * GSLOTS], BF, kind="ExternalInput")
    vm_d = nc.dram_tensor("vm", [6, P, H], BF, kind="ExternalInput")
    qv_d = nc.dram_tensor("qv", [4, H], BF, kind="ExternalInput")
    rsv_d = nc.dram_tensor("rsv", [3, cfg.npad], BF, kind="ExternalInput")
    lnm_d = nc.dram_tensor("lnm", [4, P, H], F32, kind="ExternalInput")
    pw_d = nc.dram_tensor("pw", [H, DOUT], BF, kind="ExternalInput")
    ident_d = nc.dram_tensor("ident", [P, P], BF, kind="ExternalInput")
    xst_d = nc.dram_tensor("xst", [H, cfg.npad], BF, kind="ExternalInput")
    out_d = nc.dram_tensor("out_part", [GSLOTS, DOUT], F32, kind="ExternalOutput")

    # per-AG-chunk tensors: collectives take whole tensors (no sliced APs)
    split = sched.ag_rows[0][1]                  # local rows in chunk 0
    assert cfg.cores * split == HALF
    h1sA_d = nc.dram_tensor("h1sA", [split, H], BF)
    h1sB_d = nc.dram_tensor("h1sB", [cfg.nshard - split, H], BF)
    h1fA_d = nc.dram_tensor("h1fA", [HALF, H], BF, addr_space="Shared")
    h1fB_d = nc.dram_tensor("h1fB", [cfg.N - HALF, H], BF, addr_space="Shared")

    ns = cfg.nshard
    max_tpb1 = int(sched.tpb1.max())
    max_tpb2 = int(sched.tpb2.max())

    with tile.TileContext(nc) as tc:
        with (
            tc.tile_pool(name="const", bufs=1) as cpool,
            tc.tile_pool(name="str1", bufs=3) as s1pool,
            tc.tile_pool(name="eb2", bufs=4) as e2pool,
            tc.tile_pool(name="pc2", bufs=2 * SBLK // 2 + 2) as pcpool,
            tc.tile_pool(name="z", bufs=2) as zpool,
            tc.tile_pool(name="lnt", bufs=2) as lnpool,
            tc.tile_pool(name="stat", bufs=4) as stpool,
            tc.tile_pool(name="small", bufs=4) as smpool,
            tc.tile_pool(name="ps_agg", bufs=2, space="PSUM") as ps_agg,
            tc.tile_pool(name="ps_dense", bufs=2, space="PSUM") as ps_dense,
            tc.tile_pool(name="ps_tr", bufs=2, space="PSUM") as ps_tr,
            tc.tile_pool(name="ps_pool", bufs=1, space="PSUM") as ps_pool,
        ):
            # ---------- resident constants ----------
            idx_t = cpool.tile([P, sched.idx_cols], I16)
            nc.sync.dma_start(out=idx_t[:], in_=idx_d.ap())
            epool_t = cpool.tile([P, cfg.npair * GSLOTS], BF)
            nc.sync.dma_start(out=epool_t[:], in_=epool_d.ap())
            vm_t = []
            for i in range(6):
                t = cpool.tile([P, H], BF, tag=f"vm{i}")
                nc.sync.dma_start(out=t[:], in_=vm_d.ap()[i])
                vm_t.append(t)
            ln_t = []
            for i in range(4):
                t = cpool.tile([P, H], F32, tag=f"ln{i}")
                nc.sync.dma_start(out=t[:], in_=lnm_d.ap()[i])
                ln_t.append(t)
            qv_t = []
            for i in range(4):
                t = cpool.tile([1, H], BF, tag=f"qv{i}")
                nc.sync.dma_start(out=t[:], in_=qv_d.ap()[i:i + 1, :])
                qv_t.append(t)
            rsv_t = []
            for i in range(3):
                t = cpool.tile([1, cfg.npad], BF, tag=f"rsv{i}")
                nc.sync.dma_start(out=t[:], in_=rsv_d.ap()[i:i + 1, :])
                rsv_t.append(t)
            pw_t = cpool.tile([H, DOUT], BF)
            nc.sync.dma_start(out=pw_t[:], in_=pw_d.ap())
            ident_t = cpool.tile([P, P], BF)
            nc.sync.dma_start(out=ident_t[:], in_=ident_d.ap())
            xst_t = cpool.tile([P, cfg.npad], BF)
            nc.sync.dma_start(out=xst_t[:], in_=xst_d.ap())
            h1T_t = cpool.tile([P, cfg.npad], BF)
            h1loc_t = cpool.tile([P, cfg.npair * H], BF)
            eps_t = cpool.tile([P, 1], F32)
            nc.vector.memset(eps_t[:], EPS)
            gb_ring = []
            for i in range(4):
                t = cpool.tile([P, max(max_tpb2, 1) * P], BF, tag=f"gbr{i}")
                nc.vector.memset(t[:], 0)
                gb_ring.append(t)

            pool_psum = ps_pool.tile([GSLOTS, H], F32)

            tab_lo = h1fA_d.ap()
            tab_hi = h1fB_d.ap()

            def ln_act(l, sb, z, zsrc_cols, npr, pr0):
                """Fused LayerNorm-mix + activation-mix over z[:, :npr*H]."""
                wn1 = float(sched.wn[l, 1])
                ra = float(sched.wa[l, 0] + sched.wa[l, 2])
                ta = float(sched.wa[l, 1])
                ea = float(sched.wa[l, 2])
                g_rep = ln_t[2 * l]
                b_rep = ln_t[2 * l + 1]
                have_b = sched.have_lnb[l]
                F = npr * H
                z3 = z[:, :F].rearrange("p (g c) -> p g c", c=H)
                mu = stpool.tile([P, max(npr, 1)], F32, tag="mu")
                nc.vector.tensor_reduce(out=mu[:, :npr], in_=z3,
                                        axis=mybir.AxisListType.X,
                                        op=mybir.AluOpType.add)
                nc.vector.tensor_scalar_mul(mu[:, :npr], mu[:, :npr], 1.0 / H)
                zc = lnpool.tile([P, max(npr, 1) * H], F32, tag="zc")
                nc.vector.tensor_tensor(
                    out=zc[:, :F].rearrange("p (g c) -> p g c", c=H),
                    in0=z3, in1=mu[:, :npr].to_broadcast([P, npr, H]),
                    op=mybir.AluOpType.subtract)
                sq = lnpool.tile([P, max(npr, 1) * H], F32, tag="sq")
                nc.scalar.square(out=sq[:, :F], in_=zc[:, :F])
                var = stpool.tile([P, max(npr, 1)], F32, tag="var")
                nc.vector.tensor_reduce(
                    out=var[:, :npr],
                    in_=sq[:, :F].rearrange("p (g c) -> p g c", c=H),
                    axis=mybir.AxisListType.X, op=mybir.AluOpType.add)
                sd = stpool.tile([P, max(npr, 1)], F32, tag="sd")
                nc.scalar.activation(out=sd[:, :npr], in_=var[:, :npr],
                                     func=mybir.ActivationFunctionType.Sqrt,
                                     bias=eps_t[:], scale=1.0 / H)
                rsl = stpool.tile([P, max(npr, 1)], F32, tag="rsl")
                nc.vector.reciprocal(out=rsl[:, :npr], in_=sd[:, :npr])
                u = lnpool.tile([P, max(npr, 1) * H], F32, tag="u")
                nc.vector.tensor_tensor(
                    out=u[:, :F].rearrange("p (g c) -> p g c", c=H),
                    in0=zc[:, :F].rearrange("p (g c) -> p g c", c=H),
                    in1=rsl[:, :npr].to_broadcast([P, npr, H]),
                    op=mybir.AluOpType.mult)
                g_bc = dataclasses.replace(
                    g_rep[:], ap=[g_rep[:].ap[0], [0, npr], g_rep[:].ap[1]])
                v = u
                nc.vector.tensor_tensor(
                    out=v[:, :F].rearrange("p (g c) -> p g c", c=H),
                    in0=u[:, :F].rearrange("p (g c) -> p g c", c=H),
                    in1=g_bc, op=mybir.AluOpType.mult)
                w = zc
                nc.vector.tensor_scalar_mul(w[:, :F], z[:, :F], wn1)
                hpre = u
                nc.vector.tensor_tensor(out=hpre[:, :F], in0=v[:, :F],
                                        in1=w[:, :F], op=mybir.AluOpType.add)
                if have_b:
                    b_bc = dataclasses.replace(
                        b_rep[:], ap=[b_rep[:].ap[0], [0, npr], b_rep[:].ap[1]])
                    nc.vector.tensor_tensor(
                        out=hpre[:, :F].rearrange("p (g c) -> p g c", c=H),
                        in0=hpre[:, :F].rearrange("p (g c) -> p g c", c=H),
                        in1=b_bc, op=mybir.AluOpType.add)
                th_t = sq
                nc.scalar.activation(out=th_t[:, :F], in_=hpre[:, :F],
                                     func=mybir.ActivationFunctionType.Tanh)
                m_t = w
                nc.vector.tensor_scalar_min(m_t[:, :F], hpre[:, :F], 0.0)
                e_t = z
                nc.scalar.activation(out=e_t[:, :F], in_=m_t[:, :F],
                                     func=mybir.ActivationFunctionType.Exp)
                r_t = hpre
                nc.scalar.activation(out=r_t[:, :F], in_=hpre[:, :F],
                                     func=mybir.ActivationFunctionType.Relu,
                                     scale=ra)
                nc.vector.tensor_scalar_mul(th_t[:, :F], th_t[:, :F], ta)
                nc.vector.tensor_scalar(out=e_t[:, :F], in0=e_t[:, :F],
                                        scalar1=ea, scalar2=-ea,
                                        op0=mybir.AluOpType.mult,
                                        op1=mybir.AluOpType.add)
                nc.vector.tensor_tensor(out=r_t[:, :F], in0=r_t[:, :F],
                                        in1=th_t[:, :F], op=mybir.AluOpType.add)
                return r_t, e_t

            def dense_pair(l, pr, lhs_g, lhs_s, z, prl, bias_mm):
                hsrc = xst_t if l == 0 else h1T_t
                hT_ap = hsrc[:, pr * P:(pr + 1) * P]
                po = ps_dense.tile([P, H], F32, tag="dense")
                nc.tensor.matmul(po[:], lhsT=lhs_g, rhs=vm_t[3 * l + 0][:],
                                 start=True, stop=False)
                nc.tensor.matmul(po[:], lhsT=hT_ap, rhs=vm_t[3 * l + 1][:],
                                 start=False, stop=False)
                nc.tensor.matmul(po[:], lhsT=lhs_s, rhs=vm_t[3 * l + 2][:],
                                 start=False, stop=not bias_mm)
                if bias_mm:
                    if l == 0:
                        nc.tensor.matmul(po[:], lhsT=rsv_t[0][:, pr * P:(pr + 1) * P],
                                         rhs=qv_t[0][:], start=False, stop=False)
                        nc.tensor.matmul(po[:], lhsT=rsv_t[1][:, pr * P:(pr + 1) * P],
                                         rhs=qv_t[1][:], start=False, stop=False)
                        nc.tensor.matmul(po[:], lhsT=rsv_t[2][:, pr * P:(pr + 1) * P],
                                         rhs=qv_t[2][:], start=False, stop=True)
                    else:
                        nc.tensor.matmul(po[:], lhsT=rsv_t[2][:, pr * P:(pr + 1) * P],
                                         rhs=qv_t[3][:], start=False, stop=True)
                nc.vector.tensor_copy(out=z[:, prl * H:(prl + 1) * H], in_=po[:])

            def drain_agg(ps, pr, l):
                """PSUM [g0 s0 g1 s1] -> bf16 pc2 [g0 g1 | s0 s1] via Scalar."""
                pc = pcpool.tile([P, 2 * P], BF, tag="pc2", name=f"pc2_{l}_{pr}")
                ps4 = ps[:].rearrange("p (b gs c) -> p b gs c", b=2, c=BLK)
                nc.scalar.copy(
                    out=pc[:, 0:P].rearrange("p (b c) -> p b c", c=BLK),
                    in_=ps4[:, :, 0, :])
                nc.scalar.copy(
                    out=pc[:, P:2 * P].rearrange("p (b c) -> p b c", c=BLK),
                    in_=ps4[:, :, 1, :])
                return pc

            # =================== layer 1 ===================
            ag_done = 0
            for sb in range(cfg.nsb):
                b0, b1 = sb * SBLK, min((sb + 1) * SBLK, cfg.nblk)
                npr = (b1 - b0) // 2
                pr0 = b0 // 2
                z = zpool.tile([P, max(npr, 1) * H], F32, tag="z")
                for prl in range(npr):
                    pr = pr0 + prl
                    tpb = int(sched.tpb1[pr])
                    c1 = sched.ecol1[2 * pr]
                    xgt = s1pool.tile([P, max_tpb1 * P], BF, tag="xgt",
                                      name=f"xgt_{pr}")
                    nc.sync.dma_start(out=xgt[:, :tpb * P],
                                      in_=xg_d.ap()[:, c1:c1 + tpb * P])
                    e1t = s1pool.tile([P, max_tpb1 * P], BF, tag="e1t",
                                      name=f"e1t_{pr}")
                    # GpSimd + its SWDGE queue are idle during L1: issue the
                    # E-stream loads there so they don't serialize behind the
                    # xg loads on the SP HWDGE queue
                    nc.gpsimd.dma_start(out=e1t[:, :tpb * P],
                                        in_=est1_d.ap()[:, c1:c1 + tpb * P])
                    ps = ps_agg.tile([P, 2 * P], F32, tag="agg")
                    toff = 0
                    for side in (0, 1):
                        b = 2 * pr + side
                        nt = int(sched.T1[b])
                        for k in range(nt):
                            kk = toff + k
                            nc.tensor.matmul(
                                ps[:, side * P:(side + 1) * P],
                                lhsT=xgt[:, kk * P:(kk + 1) * P],
                                rhs=e1t[:, kk * P:(kk + 1) * P],
                                start=(k == 0), stop=(k == nt - 1))
                        toff += nt
                    pc = drain_agg(ps, pr, 0)
                    dense_pair(0, pr, pc[:, 0:P], pc[:, P:2 * P], z, prl,
                               sched.have_bias1)
                r_t, e_t = ln_act(0, sb, z, None, npr, pr0)
                F = npr * H
                hdst = h1loc_t[:, pr0 * H:pr0 * H + F]
                nc.vector.tensor_tensor(out=hdst, in0=r_t[:, :F], in1=e_t[:, :F],
                                        op=mybir.AluOpType.add)
                for prl in range(npr):
                    pr = pr0 + prl
                    rows = min(P, ns - pr * P)
                    if rows > 0:
                        if pr * P < split:
                            hs_ap = h1sA_d.ap()[pr * P:pr * P + rows, :]
                        else:
                            r0_ = pr * P - split
                            hs_ap = h1sB_d.ap()[r0_:r0_ + rows, :]
                        nc.sync.dma_start(
                            out=hs_ap,
                            in_=h1loc_t[0:rows, pr * H:(pr + 1) * H])
                    pt = ps_tr.tile([P, P], BF, tag="tr")
                    nc.tensor.transpose(out=pt[:],
                                        in_=h1loc_t[:, pr * H:(pr + 1) * H],
                                        identity=ident_t[:])
                    nc.vector.tensor_copy(out=h1T_t[:, pr * P:(pr + 1) * P],
                                          in_=pt[:])
                # chunked AllGather as soon as this chunk's rows are stored
                if ag_done < len(AG_SB_END) and sb == AG_SB_END[ag_done]:
                    cin = h1sA_d if ag_done == 0 else h1sB_d
                    cout = h1fA_d if ag_done == 0 else h1fB_d
                    nc.gpsimd.collective_compute(
                        "AllGather", mybir.AluOpType.bypass,
                        replica_groups=[list(range(cfg.cores))],
                        ins=[cin.ap()], outs=[cout.ap()])
                    ag_done += 1

            # =================== layer 2 ===================
            RING = len(gb_ring)
            WARM = 2
            PPSB = SBLK // 2

            def issue_gathers(pr, half):
                gb = gb_ring[pr % RING]
                for s in sched.buckets[pr]:
                    if s["h"] != half:
                        continue
                    toff, ntl = s["tiles"]
                    n = s["n"]
                    tabn = tab_lo if s["h"] == 0 else tab_hi
                    nc.gpsimd.dma_gather(
                        out_ap=gb[:, toff * P:(toff + ntl) * P]
                        .rearrange("p (t c) -> p t c", c=P),
                        in_ap=tabn,
                        idxs_ap=idx_t[:, s["io"] // 16:(s["io"] + n) // 16],
                        num_idxs=n, num_idxs_reg=n, elem_size=H,
                        queue_num=s["q"])

            for pr in range(min(WARM, cfg.npair)):
                issue_gathers(pr, 0)
            z = None
            for pr in range(cfg.npair):
                sb = pr // PPSB
                prl = pr % PPSB
                npr = min(PPSB, cfg.npair - sb * PPSB)
                pr0 = sb * PPSB
                if prl == 0:
                    z = zpool.tile([P, max(npr, 1) * H], F32, tag="z")
                if pr + WARM < cfg.npair:
                    issue_gathers(pr + WARM, 0)
                issue_gathers(pr, 1)
                tpb = int(sched.tpb2[pr])
                c2 = sched.ecol2[pr]
                e2t = e2pool.tile([P, max_tpb2 * P], BF, tag="e2t",
                                  name=f"e2t_{pr}")
                # Activation-engine HWDGE queue: keeps est2 issue off the SP
                # queue while SWDGE queues drain the gathers
                nc.scalar.dma_start(out=e2t[:, :tpb * P],
                                    in_=est2_d.ap()[:, c2:c2 + tpb * P])
                gb = gb_ring[pr % RING]
                ps = ps_agg.tile([P, 2 * P], F32, tag="agg")
                chunks = sched.mmt[pr]
                tot_side = [sum(1 for c_ in chunks if c_[0] == 0),
                            sum(1 for c_ in chunks if c_[0] == 1)]
                done = [0, 0]
                for (side, kk, r0, r1) in chunks:
                    nc.tensor.matmul(
                        ps[:, side * P:(side + 1) * P],
                        lhsT=gb[r0:r1, kk * P:(kk + 1) * P],
                        rhs=e2t[r0:r1, kk * P:(kk + 1) * P],
                        start=(done[side] == 0),
                        stop=(done[side] == tot_side[side] - 1))
                    done[side] += 1
                pc = drain_agg(ps, pr, 1)
                dense_pair(1, pr, pc[:, 0:P], pc[:, P:2 * P], z, prl,
                           sched.have_bias2)
                if prl == npr - 1:
                    r_t, e_t = ln_act(1, sb, z, None, npr, pr0)
                    F = npr * H
                    h2sb = lnpool.tile([P, max(npr, 1) * H], BF, tag="h2")
                    nc.vector.tensor_tensor(out=h2sb[:, :F], in0=r_t[:, :F],
                                            in1=e_t[:, :F],
                                            op=mybir.AluOpType.add)
                    skip = h2sb
                    nc.vector.tensor_tensor(
                        out=skip[:, :F],
                        in0=h1loc_t[:, pr0 * H:pr0 * H + F],
                        in1=h2sb[:, :F], op=mybir.AluOpType.add)
                    for pl in range(npr):
                        pp = pr0 + pl
                        nc.tensor.matmul(
                            pool_psum[:],
                            lhsT=epool_t[:, pp * GSLOTS:(pp + 1) * GSLOTS],
                            rhs=skip[:, pl * H:(pl + 1) * H],
                            start=(pp == 0), stop=(pp == cfg.npair - 1))

            # ---------- readout: pooled @ post_w ----------
            poolc = smpool.tile([GSLOTS, H], BF, tag="poolc")
            nc.vector.tensor_copy(out=poolc[:], in_=pool_psum[:])
            pt = ps_tr.tile([P, GSLOTS], BF, tag="tr")
            nc.tensor.transpose(out=pt[:], in_=poolc[:], identity=ident_t[:])
            ptc = smpool.tile([P, GSLOTS], BF, tag="ptc")
            nc.vector.tensor_copy(out=ptc[:], in_=pt[:])
            ops = ps_dense.tile([GSLOTS, DOUT], F32, tag="dense")
            nc.tensor.matmul(ops[:], lhsT=ptc[:], rhs=pw_t[:], start=True, stop=True)
            outc = smpool.tile([GSLOTS, DOUT], F32, tag="outc")
            nc.vector.tensor_copy(out=outc[:], in_=ops[:])
            nc.sync.dma_start(out=out_d.ap(), in_=outc[:])

    nc.compile()
    return nc


def _kernel_impl(inputs: dict, cfg: Cfg = None, trace: bool = False):
    if cfg is None:
        cfg = Cfg(N=50000, E=640000, G=500, cores=8)
    sched, data, combine = host_prep(inputs, cfg)
    nc = build_program(cfg, sched)
    in_maps = [data[c] for c in range(cfg.cores)]
    res = run_bass_kernel_spmd(nc, in_maps, core_ids=list(range(cfg.cores)),
                               trace=trace)
    out = np.zeros((cfg.G, DOUT), np.float64)
    for c in range(cfg.cores):
        part = np.asarray(res.results[c]["out_part"], np.float64)
        lo = combine["g_lo"][c]
        hi = min(lo + GSLOTS, cfg.G)
        out[lo:hi] += part[:hi - lo]
    out += combine["post_b"]
    return out.astype(np.float32), res


def kernel(**inputs) -> np.ndarray:
    out, _ = _kernel_impl(inputs)
    return out
